# revision 1
# baseline (speedup 1.0000x reference)
"""Distributed 2-layer GCN (DGL GraphConv norm='both') + mean-pool head on 8 TRN2 NeuronCores.

Strategy
--------
GraphConv is linear, so fold both degree normalizations into per-edge weights
w_e = rsqrt(deg_out[src_e]) * rsqrt(deg_in[dst_e]) and reorder each layer as
transform-then-aggregate:

    t = x @ W                       (dense matmul, node-sharded across cores)
    agg[d] = sum_e w_e * t[src_e]   (sparse aggregation, dst-sharded)
    feat = relu(agg + b)            (leaky_relu after relu is a no-op)

Sharding: nodes are split into 8 contiguous shards; core k computes t-rows and
aggregations for its shard. After each transform the t-shards are AllGathered
so every core can gather arbitrary source rows.

Aggregation on device: edges are sorted by dst and grouped into 128-dst-node
blocks (host-side, index-only preprocessing). For each block, source rows are
fetched with one indirect DMA (row gather), and the scatter-add becomes a
dense matmul: for each chunk of 128 edges, build S[e, slot] =
(iota[slot] == dst_slot_e) * w_e with a single fused tensor_scalar op, then
PSUM-accumulate G_chunk.T @ S -> agg^T [feat, dst_slot]. The transposed layout
makes the bias a per-partition ACT bias and feeds the next layer's matmul
(lhsT = relu(agg^T + b)) with no transposes anywhere.

Readout: ACT accum_out gives the free-axis (dst) sum of relu() for free;
per-core partials are AllReduced, and the tiny MLP head runs in column form
(out = W.T @ col) on every core redundantly.
"""

import sys

sys.path.insert(0, "/opt/trn_rl_repo")

import numpy as np

import concourse.bacc as bacc
import concourse.bass as bass
import concourse.mybir as mybir
import concourse.tile as tile
from concourse.bass import IndirectOffsetOnAxis
from concourse.bass_utils import run_bass_kernel_spmd

NCORES = 8
P = 128
CPAD = 16
LEAKY = 0.01
F32 = mybir.dt.float32
BF16 = mybir.dt.bfloat16
I32 = mybir.dt.int32


def _dims(N, E):
    assert N % NCORES == 0
    shard = N // NCORES
    nblk = -(-shard // P)
    shard_pad = nblk * P
    bt = max(d for d in range(1, 33) if nblk % d == 0)
    valid_last = shard - (nblk - 1) * P
    return shard, nblk, shard_pad, bt, valid_last


def preprocess(x, src, dst):
    """Index-only host preprocessing: degree rsqrt folding, edge partitioning
    by (dst-shard, dst-block), uniform chunk padding, per-core input maps."""
    N, D = x.shape
    E = src.shape[0]
    assert D == P
    shard, nblk, shard_pad, bt, valid_last = _dims(N, E)

    src = np.asarray(src).astype(np.int64)
    dst = np.asarray(dst).astype(np.int64)

    deg_out = np.bincount(src, minlength=N).astype(np.float32)
    deg_in = np.bincount(dst, minlength=N).astype(np.float32)
    r_out = (1.0 / np.sqrt(np.maximum(deg_out, 1.0))).astype(np.float32)
    r_in = (1.0 / np.sqrt(np.maximum(deg_in, 1.0))).astype(np.float32)

    order = np.argsort(dst, kind="stable")
    ds = dst[order]
    ss = src[order]

    gid = (ds // shard) * nblk + (ds % shard) // P
    ngrp = NCORES * nblk
    counts = np.bincount(gid, minlength=ngrp)
    K1 = max(1, int(-(-counts.max() // P)))
    CAP = K1 * P

    starts = np.zeros(ngrp, np.int64)
    starts[1:] = np.cumsum(counts)[:-1]
    pos = np.arange(E, dtype=np.int64) - starts[gid]
    flat = gid * CAP + pos

    pid = ((ss // shard) * shard_pad + (ss % shard)).astype(np.int32)
    idx_flat = np.zeros(ngrp * CAP, np.int32)
    slot_flat = np.zeros(ngrp * CAP, np.float32)
    w_flat = np.zeros(ngrp * CAP, np.float32)
    idx_flat[flat] = pid
    slot_flat[flat] = ((ds % shard) % P).astype(np.float32)
    w_flat[flat] = r_out[ss] * r_in[ds]

    idx_a = idx_flat.reshape(NCORES, nblk, P, K1)
    slot_a = slot_flat.reshape(NCORES, nblk, P, K1)
    w_a = w_flat.reshape(NCORES, nblk, P, K1)

    # per-core transposed x shard, zero-padded to shard_pad columns
    xT = np.zeros((NCORES, P, shard_pad), np.float32)
    xv = np.ascontiguousarray(x.astype(np.float32))
    for k in range(NCORES):
        xT[k, :, :shard] = xv[k * shard : (k + 1) * shard].T

    iota = np.tile(np.arange(P, dtype=np.float32), (P, 1))
    return dict(
        N=N, E=E, shard=shard, nblk=nblk, shard_pad=shard_pad, bt=bt,
        valid_last=valid_last, K1=K1, xT=xT, idx=idx_a, slot=slot_a, w=w_a,
        iota=iota,
    )


def build_nc(N, nblk, shard_pad, bt, valid_last, K1):
    """Build the SPMD Bass program (same program for all 8 cores)."""
    CAP = K1 * P
    rg = [list(range(NCORES))]
    nc = bacc.Bacc("TRN2", target_bir_lowering=False, debug=False,
                   num_devices=NCORES)

    xT_p = nc.declare_dram_parameter("xT", [P, shard_pad], F32, False)
    w0_p = nc.declare_dram_parameter("W0", [P, P], F32, False)
    w1_p = nc.declare_dram_parameter("W1", [P, P], F32, False)
    wl1_p = nc.declare_dram_parameter("WL1", [P, P], F32, False)
    wl2_p = nc.declare_dram_parameter("WL2", [P, CPAD], F32, False)
    b0_p = nc.declare_dram_parameter("b0", [P, 1], F32, False)
    b1_p = nc.declare_dram_parameter("b1", [P, 1], F32, False)
    bl1_p = nc.declare_dram_parameter("bL1", [P, 1], F32, False)
    bl2_p = nc.declare_dram_parameter("bL2", [CPAD, 1], F32, False)
    iota_p = nc.declare_dram_parameter("iota", [P, P], F32, False)
    idx_p = nc.declare_dram_parameter("idx", [nblk, P, K1], I32, False)
    slot_p = nc.declare_dram_parameter("slot", [nblk, P, K1], F32, False)
    wgt_p = nc.declare_dram_parameter("wgt", [nblk, P, K1], F32, False)
    y_p = nc.declare_dram_parameter("y", [CPAD, 1], F32, True)

    with tile.TileContext(nc) as tc:
        with (
            tc.tile_pool(name="consts", bufs=1) as consts,
            tc.tile_pool(name="xin", bufs=2) as xin,
            tc.tile_pool(name="stg", bufs=3) as stg,
            tc.tile_pool(name="auxp", bufs=3) as auxp,
            tc.tile_pool(name="gp", bufs=2) as gp,
            tc.tile_pool(name="sp", bufs=4) as sp,
            tc.tile_pool(name="hp", bufs=2) as hp,
            tc.tile_pool(name="misc", bufs=1) as misc,
            tc.tile_pool(name="psA", bufs=2, space="PSUM") as psA,
            tc.tile_pool(name="psB", bufs=2, space="PSUM") as psB,
            tc.tile_pool(name="dram", bufs=1, space="DRAM") as dram,
        ):
            # ---- constants ----
            w0bf = consts.tile([P, P], BF16)
            nc.gpsimd.dma_start(w0bf[:], w0_p[:])  # f32 -> bf16 cast DMA
            w1bf = consts.tile([P, P], BF16)
            nc.gpsimd.dma_start(w1bf[:], w1_p[:])
            wl1sb = consts.tile([P, P], F32)
            nc.sync.dma_start(wl1sb[:], wl1_p[:])
            wl2sb = consts.tile([P, CPAD], F32)
            nc.sync.dma_start(wl2sb[:], wl2_p[:])
            b0c = consts.tile([P, 1], F32)
            nc.sync.dma_start(b0c[:], b0_p[:])
            b1c = consts.tile([P, 1], F32)
            nc.sync.dma_start(b1c[:], b1_p[:])
            bl1c = consts.tile([P, 1], F32)
            nc.sync.dma_start(bl1c[:], bl1_p[:])
            bl2c = consts.tile([CPAD, 1], F32)
            nc.sync.dma_start(bl2c[:], bl2_p[:])
            iota_f = consts.tile([P, P], F32)
            nc.sync.dma_start(iota_f[:], iota_p[:])
            iota_sb = consts.tile([P, P], BF16)
            nc.vector.tensor_copy(iota_sb[:], iota_f[:])

            t0loc = dram.tile([shard_pad, P], BF16)
            t0full = dram.tile([NCORES * shard_pad, P], BF16, addr_space="Shared")
            t1loc = dram.tile([shard_pad, P], BF16)
            t1full = dram.tile([NCORES * shard_pad, P], BF16, addr_space="Shared")
            arin = dram.tile([P, 1], F32)
            arout = dram.tile([P, 1], F32, addr_space="Shared")

            # ---- phase A: t0 shard = x_shard @ W0 (bf16) ----
            for t in range(nblk // bt):
                xsb = xin.tile([P, bt * P], BF16, tag="xsb")
                nc.gpsimd.dma_start(
                    xsb[:], xT_p[:, t * bt * P : (t + 1) * bt * P]
                )
                for i in range(bt):
                    b = t * bt + i
                    pt0 = psA.tile([P, P], F32, space="PSUM", tag="pt0")
                    nc.tensor.matmul(
                        pt0[:], lhsT=xsb[:, i * P : (i + 1) * P], rhs=w0bf[:],
                        start=True, stop=True,
                    )
                    st = stg.tile([P, P], BF16, tag="st")
                    nc.vector.tensor_copy(st[:], pt0[:])
                    nc.sync.dma_start(t0loc[b * P : (b + 1) * P, :], st[:])

            nc.gpsimd.collective_compute(
                "AllGather", mybir.AluOpType.bypass, replica_groups=rg,
                ins=[t0loc.opt()], outs=[t0full.opt()],
            )

            def agg_layer(tfull, bias_col, produce, acc_tile):
                """Aggregate over this core's dst blocks, gathering from tfull.
                produce=True: write relu-layer t-transform into t1loc.
                produce=False: accumulate readout sums into acc_tile."""
                for b in range(nblk):
                    isb = auxp.tile([P, K1], I32, tag="isb")
                    nc.sync.dma_start(isb[:], idx_p[b])
                    ssb = auxp.tile([P, K1], F32, tag="ssb")
                    nc.sync.dma_start(ssb[:], slot_p[b])
                    wsb = auxp.tile([P, K1], F32, tag="wsb")
                    nc.sync.dma_start(wsb[:], wgt_p[b])
                    G = gp.tile([P, CAP], BF16, tag="G")
                    pagg = psB.tile([P, P], F32, space="PSUM", tag="pagg")
                    for j in range(K1):
                        nc.gpsimd.indirect_dma_start(
                            out=G[:, j * P : (j + 1) * P], out_offset=None,
                            in_=tfull,
                            in_offset=IndirectOffsetOnAxis(
                                ap=isb[:, j : j + 1], axis=0
                            ),
                        )
                        S = sp.tile([P, P], BF16, tag="S")
                        nc.vector.tensor_scalar(
                            out=S[:], in0=iota_sb[:],
                            scalar1=ssb[:, j : j + 1], scalar2=wsb[:, j : j + 1],
                            op0=mybir.AluOpType.is_equal, op1=mybir.AluOpType.mult,
                        )
                        nc.tensor.matmul(
                            pagg[:], lhsT=G[:, j * P : (j + 1) * P], rhs=S[:],
                            start=(j == 0), stop=(j == K1 - 1),
                        )
                    if produce:
                        h1 = hp.tile([P, P], BF16, tag="h1")
                        nc.scalar.activation(
                            out=h1[:], in_=pagg[:],
                            func=mybir.ActivationFunctionType.Relu, bias=bias_col,
                        )
                        pt1 = psA.tile([P, P], F32, space="PSUM", tag="pt0")
                        nc.tensor.matmul(
                            pt1[:], lhsT=h1[:], rhs=w1bf[:], start=True, stop=True
                        )
                        st1 = stg.tile([P, P], BF16, tag="st")
                        nc.vector.tensor_copy(st1[:], pt1[:])
                        nc.sync.dma_start(t1loc[b * P : (b + 1) * P, :], st1[:])
                    else:
                        nv = valid_last if b == nblk - 1 else P
                        dead = hp.tile([P, P], BF16, tag="h1")
                        nc.scalar.activation(
                            out=dead[:, :nv], in_=pagg[:, :nv],
                            func=mybir.ActivationFunctionType.Relu, bias=bias_col,
                            accum_out=acc_tile[:, b : b + 1],
                        )

            agg_layer(t0full[:, :], b0c[:, 0:1], True, None)

            nc.gpsimd.collective_compute(
                "AllGather", mybir.AluOpType.bypass, replica_groups=rg,
                ins=[t1loc.opt()], outs=[t1full.opt()],
            )

            acc = misc.tile([P, nblk], F32)
            agg_layer(t1full[:, :], b1c[:, 0:1], False, acc)

            # ---- readout: mean-pool + tiny MLP head (column form) ----
            partial = misc.tile([P, 1], F32)
            nc.vector.tensor_reduce(
                out=partial[:], in_=acc[:], axis=mybir.AxisListType.X,
                op=mybir.AluOpType.add,
            )
            nc.sync.dma_start(arin[:], partial[:])
            nc.gpsimd.collective_compute(
                "AllReduce", mybir.AluOpType.add, replica_groups=rg,
                ins=[arin.opt()], outs=[arout.opt()],
            )
            mr = misc.tile([P, 1], F32)
            nc.sync.dma_start(mr[:], arout[:])
            mc = misc.tile([P, 1], F32)
            nc.vector.tensor_scalar_mul(mc[:], mr[:], 1.0 / float(N))
            ph = psB.tile([P, 1], F32, space="PSUM", tag="ph")
            nc.tensor.matmul(ph[:], lhsT=wl1sb[:], rhs=mc[:], start=True, stop=True)
            z = misc.tile([P, 1], F32)
            nc.vector.tensor_scalar(
                out=z[:], in0=ph[:], scalar1=bl1c[:, 0:1], scalar2=None,
                op0=mybir.AluOpType.add,
            )
            za = misc.tile([P, 1], F32)
            nc.vector.tensor_scalar_mul(za[:], z[:], LEAKY)
            hg = misc.tile([P, 1], F32)
            nc.vector.tensor_tensor(
                out=hg[:], in0=z[:], in1=za[:], op=mybir.AluOpType.max
            )
            po = psB.tile([P, 1], F32, space="PSUM", tag="ph")
            nc.tensor.matmul(
                po[:CPAD, :], lhsT=wl2sb[:], rhs=hg[:], start=True, stop=True
            )
            yv = misc.tile([CPAD, 1], F32)
            nc.vector.tensor_scalar(
                out=yv[:], in0=po[:CPAD, :], scalar1=bl2c[:, 0:1], scalar2=None,
                op0=mybir.AluOpType.add,
            )
            nc.sync.dma_start(y_p[:], yv[:])

    nc.compile()
    return nc


def make_in_maps(hd, W0, b0, W1, b1, WL1, bL1, WL2, bL2):
    C = WL2.shape[1]
    wl2p = np.zeros((P, CPAD), np.float32)
    wl2p[:, :C] = np.asarray(WL2, np.float32)
    bl2c = np.zeros((CPAD, 1), np.float32)
    bl2c[:C, 0] = np.asarray(bL2, np.float32)
    shared = dict(
        W0=np.asarray(W0, np.float32), W1=np.asarray(W1, np.float32),
        WL1=np.asarray(WL1, np.float32), WL2=wl2p,
        b0=np.asarray(b0, np.float32).reshape(P, 1),
        b1=np.asarray(b1, np.float32).reshape(P, 1),
        bL1=np.asarray(bL1, np.float32).reshape(P, 1), bL2=bl2c,
        iota=hd["iota"],
    )
    return [
        dict(shared, xT=hd["xT"][k], idx=hd["idx"][k], slot=hd["slot"][k],
             wgt=hd["w"][k])
        for k in range(NCORES)
    ]


def kernel(x, src, dst, W0, b0, W1, b1, WL1, bL1, WL2, bL2):
    x = np.asarray(x)
    hd = preprocess(x, np.asarray(src), np.asarray(dst))
    nc = build_nc(hd["N"], hd["nblk"], hd["shard_pad"], hd["bt"],
                  hd["valid_last"], hd["K1"])
    in_maps = make_in_maps(hd, W0, b0, W1, b1, WL1, bL1, WL2, bL2)
    res = run_bass_kernel_spmd(nc, in_maps, list(range(NCORES)))
    C = np.asarray(WL2).shape[1]
    return res.results[0]["y"][:C, 0].reshape(1, C).astype(np.float32)



# revision 10
# speedup vs baseline: 1.0448x; 1.0448x over previous
"""Distributed 2-layer GCN (DGL GraphConv norm='both') + mean-pool head on 8 TRN2 NeuronCores.

Strategy
--------
GraphConv is linear, so fold both degree normalizations into per-edge weights
w_e = rsqrt(deg_out[src_e]) * rsqrt(deg_in[dst_e]) and reorder each layer as
transform-then-aggregate:

    t = x @ W                       (dense matmul, node-sharded across cores)
    agg[d] = sum_e w_e * t[src_e]   (sparse aggregation, dst-sharded)
    feat = relu(agg + b)            (leaky_relu after relu is a no-op)

Sharding: nodes are split into 8 contiguous shards; core k computes t-rows and
aggregations for its shard. After each transform the t-shards are AllGathered
so every core can gather arbitrary source rows.

Aggregation on device: edges are sorted by dst and grouped into 128-dst-node
blocks (host-side, index-only preprocessing). Source rows are fetched with
BULK dma_gather (the SWDGE gather custom-DMA: thousands of int16 indices per
call, 256B rows) instead of per-chunk indirect DMAs. Because gather indices
are int16 (< 32768), the 100352-row t-table is split into 4 pieces of 25088
rows; each (dst-block, piece) edge segment is padded to a multiple of 128 and
gathers are batched over groups of 4 dst blocks -> ~100 gather calls per
layer per core instead of ~3300 indirect DMAs. Within each segment edges are
sorted by source row so the random 256B reads walk ascending addresses.

For each chunk of 128 edges, the scatter-add becomes a dense matmul: build
S[e, slot] = (iota[slot] == dst_slot_e) * w_e with one fused tensor_scalar op,
then PSUM-accumulate G_chunk.T @ S -> agg^T [feat, dst_slot]. The transposed
layout makes the bias a per-partition ACT bias and feeds the next layer's
matmul (lhsT = relu(agg^T + b)) with no transposes anywhere.

Readout: ACT accum_out gives the free-axis (dst) sum of relu() for free;
per-core partials are AllReduced, and the tiny MLP head runs in column form
(out = W.T @ col) on every core redundantly.
"""

import sys

sys.path.insert(0, "/opt/trn_rl_repo")

import numpy as np
import ml_dtypes

import concourse.bacc as bacc
import concourse.bass as bass
import concourse.mybir as mybir
import concourse.tile as tile
from concourse.bass_utils import run_bass_kernel_spmd

NCORES = 8
P = 128
CPAD = 16
PIECES = 4
BGS = 2  # dst blocks per gather batch
LEAKY = 0.01
F32 = mybir.dt.float32
BF16 = mybir.dt.bfloat16
I16 = mybir.dt.int16


def preprocess(x, src, dst):
    """Index-only host preprocessing: degree rsqrt folding, edge partitioning
    by (dst-core, dst-block, src-piece), per-(block,piece) padding to 128
    multiples (shared caps across cores for SPMD), per-core input streams."""
    N, D = x.shape
    E = src.shape[0]
    assert D == P and N % NCORES == 0
    shard = N // NCORES
    nblk = -(-shard // P)
    shard_pad = nblk * P
    valid_last = shard - (nblk - 1) * P
    TAB = NCORES * shard_pad
    assert TAB % PIECES == 0
    prows = TAB // PIECES
    assert prows <= 32768

    src = np.asarray(src).astype(np.int64)
    dst = np.asarray(dst).astype(np.int64)

    deg_out = np.bincount(src, minlength=N).astype(np.float32)
    deg_in = np.bincount(dst, minlength=N).astype(np.float32)
    r_out = (1.0 / np.sqrt(np.maximum(deg_out, 1.0))).astype(np.float32)
    r_in = (1.0 / np.sqrt(np.maximum(deg_in, 1.0))).astype(np.float32)

    pid = ((src // shard) * shard_pad + (src % shard)).astype(np.int64)
    core = dst // shard
    local = dst % shard
    block = local // P
    slot = (local % P).astype(np.float32)
    piece = pid // prows
    idxrel = (pid % prows).astype(np.int64)
    w = (r_out[src] * r_in[dst]).astype(np.float32)

    # sort edges by (core, block, piece, idxrel)
    key = ((core * nblk + block) * PIECES + piece) * np.int64(prows) + idxrel
    order = np.argsort(key, kind="stable")
    core_s = core[order]
    gkey = (core * nblk + block) * PIECES + piece
    counts = np.bincount(gkey, minlength=NCORES * nblk * PIECES).reshape(
        NCORES, nblk, PIECES
    )
    cap = (-(-counts.max(axis=0) // P) * P).astype(np.int64)  # [nblk, PIECES]
    assert (counts.max(axis=0) <= cap).all()

    # block groups
    groups = [list(range(i, min(i + BGS, nblk))) for i in range(0, nblk, BGS)]

    # stream layout per core: for bg: for q: for b in bg: segment cap[b][q]
    seg_off = np.zeros((nblk, PIECES), np.int64)
    pos = 0
    for blocks in groups:
        for q in range(PIECES):
            for b in blocks:
                seg_off[b, q] = pos
                pos += cap[b, q]
    TOT = pos
    assert TOT % P == 0
    TOTCH = TOT // P

    # rank of each edge within its (core, block, piece) segment
    gkey_s = gkey[order]
    # edges sorted by key -> within-segment rank = arange - segment start
    seg_starts_sorted = np.zeros(NCORES * nblk * PIECES, np.int64)
    cnt_flat = np.bincount(gkey_s, minlength=NCORES * nblk * PIECES)
    seg_starts_sorted[1:] = np.cumsum(cnt_flat)[:-1]
    rank = np.arange(E, dtype=np.int64) - seg_starts_sorted[gkey_s]
    b_s = block[order]
    q_s = piece[order]
    stream_pos = seg_off[b_s, q_s] + rank  # position within the core's stream

    idx_stream = np.zeros((NCORES, TOT), np.int16)
    slot_stream = np.zeros((NCORES, TOT), np.float32)
    w_stream = np.zeros((NCORES, TOT), np.float32)
    idx_stream[core_s, stream_pos] = idxrel[order].astype(np.int16)
    slot_stream[core_s, stream_pos] = slot[order]
    w_stream[core_s, stream_pos] = w[order]

    # gidx: wrap each call segment [16, len/16] -> replicate x8 -> [128, ...]
    # wrapped element g -> [g%16, g//16]; since every call segment length is a
    # multiple of 128 (hence 16), wrapping the whole stream at once is
    # identical to wrapping per call.
    gidx = np.zeros((NCORES, 128, TOT // 16), np.int16)
    w16 = idx_stream.reshape(NCORES, TOT // 16, 16).transpose(0, 2, 1)
    gidx[:] = np.tile(w16, (1, 8, 1))

    # per-chunk slot/w arrays [128, TOTCH]
    sarr = slot_stream.reshape(NCORES, TOTCH, P).transpose(0, 2, 1)
    warr = w_stream.reshape(NCORES, TOTCH, P).transpose(0, 2, 1)
    # gsw layout: per bg: [slot chunks | w chunks]
    sw_parts = []
    ch_off = {}
    c = 0
    for gi, blocks in enumerate(groups):
        nch = sum(int(cap[b, q]) for b in blocks for q in range(PIECES)) // P
        ch_off[gi] = c
        sw_parts.append(sarr[:, :, c : c + nch])
        sw_parts.append(warr[:, :, c : c + nch])
        c += nch
    assert c == TOTCH
    gsw = np.concatenate(sw_parts, axis=2)

    xT = np.zeros((NCORES, P, shard_pad), ml_dtypes.bfloat16)
    xv = np.ascontiguousarray(np.asarray(x, np.float32))
    for k in range(NCORES):
        xT[k, :, :shard] = xv[k * shard : (k + 1) * shard].T.astype(
            ml_dtypes.bfloat16)

    iota = np.tile(np.arange(P, dtype=np.float32), (P, 1)).astype(
        ml_dtypes.bfloat16)

    bt = max(d for d in range(1, 33) if nblk % d == 0)

    meta = dict(
        N=N, E=E, shard=shard, nblk=nblk, shard_pad=shard_pad, bt=bt,
        valid_last=valid_last, TAB=TAB, prows=prows, TOT=TOT, TOTCH=TOTCH,
        cap=cap, groups=groups, ch_off=ch_off,
    )
    return dict(meta=meta, xT=xT, gidx=gidx, gsw=gsw, iota=iota)


def build_nc(meta):
    """Build the SPMD Bass program (same program for all 8 cores)."""
    nblk = meta["nblk"]
    shard_pad = meta["shard_pad"]
    bt = meta["bt"]
    valid_last = meta["valid_last"]
    TAB = meta["TAB"]
    prows = meta["prows"]
    TOT = meta["TOT"]
    TOTCH = meta["TOTCH"]
    cap = meta["cap"]
    groups = meta["groups"]
    ch_off = meta["ch_off"]
    N = meta["N"]

    # per-call capacities and offsets
    call_cap = {}  # (gi, q) -> num idxs
    call_coloff = {}  # (gi, q) -> col offset into gidx
    bg_coloff = {}  # gi -> (col0, ncols)
    pos = 0
    for gi, blocks in enumerate(groups):
        c0 = pos // 16
        for q in range(PIECES):
            call_cap[(gi, q)] = sum(int(cap[b, q]) for b in blocks)
            call_coloff[(gi, q)] = (pos // 16) - c0
            pos += call_cap[(gi, q)]
        bg_coloff[gi] = (c0, pos // 16 - c0)
    GMAX = max(call_cap.values())
    IDXW = max(n for _, n in bg_coloff.values())
    NCHW = max(
        sum(int(cap[b, q]) for b in blocks for q in range(PIECES)) // P
        for blocks in groups
    )

    rg = [list(range(NCORES))]
    nc = bacc.Bacc("TRN2", target_bir_lowering=False, debug=False,
                   num_devices=NCORES, dynamic_dma_scratch_size=32768,
                   num_swdge_queues=4)

    xT_p = nc.declare_dram_parameter("xT", [P, shard_pad], BF16, False)
    w0_p = nc.declare_dram_parameter("W0", [P, P], BF16, False)
    w1_p = nc.declare_dram_parameter("W1", [P, P], BF16, False)
    wl1_p = nc.declare_dram_parameter("WL1", [P, P], F32, False)
    wl2_p = nc.declare_dram_parameter("WL2", [P, CPAD], F32, False)
    b0_p = nc.declare_dram_parameter("b0", [P, 1], F32, False)
    b1_p = nc.declare_dram_parameter("b1", [P, 1], F32, False)
    bl1_p = nc.declare_dram_parameter("bL1", [P, 1], F32, False)
    bl2_p = nc.declare_dram_parameter("bL2", [CPAD, 1], F32, False)
    iota_p = nc.declare_dram_parameter("iota", [P, P], BF16, False)
    gidx_p = nc.declare_dram_parameter("gidx", [128, TOT // 16], I16, False)
    gsw_p = nc.declare_dram_parameter("gsw", [128, 2 * TOTCH], F32, False)
    y_p = nc.declare_dram_parameter("y", [CPAD, 1], F32, True)

    _sids = {}

    def scope_in(name):
        _sids[name] = nc.enter_named_scope(name, False)[0]

    def scope_out(name):
        nc.leave_named_scope(name, _sids[name], False)

    with tile.TileContext(nc) as tc:
        with (
            tc.tile_pool(name="consts", bufs=1) as consts,
            tc.tile_pool(name="xin", bufs=2) as xin,
            tc.tile_pool(name="stg", bufs=3) as stg,
            tc.tile_pool(name="idxp", bufs=2) as idxp,
            tc.tile_pool(name="swp", bufs=2) as swp,
            tc.tile_pool(name="gp", bufs=3) as gp,
            tc.tile_pool(name="sp", bufs=4) as sp,
            tc.tile_pool(name="hp", bufs=2) as hp,
            tc.tile_pool(name="misc", bufs=1) as misc,
            tc.tile_pool(name="psA", bufs=2, space="PSUM") as psA,
            tc.tile_pool(name="psB", bufs=4, space="PSUM") as psB,
            tc.tile_pool(name="psH", bufs=2, space="PSUM") as psH,
            tc.tile_pool(name="dram", bufs=1, space="DRAM") as dram,
        ):
            # ---- constants ----
            w0bf = consts.tile([P, P], BF16)
            nc.sync.dma_start(w0bf[:], w0_p[:])
            w1bf = consts.tile([P, P], BF16)
            nc.sync.dma_start(w1bf[:], w1_p[:])
            wl1sb = consts.tile([P, P], F32)
            nc.sync.dma_start(wl1sb[:], wl1_p[:])
            wl2sb = consts.tile([P, CPAD], F32)
            nc.sync.dma_start(wl2sb[:], wl2_p[:])
            b0c = consts.tile([P, 1], F32)
            nc.sync.dma_start(b0c[:], b0_p[:])
            b1c = consts.tile([P, 1], F32)
            nc.sync.dma_start(b1c[:], b1_p[:])
            bl1c = consts.tile([P, 1], F32)
            nc.sync.dma_start(bl1c[:], bl1_p[:])
            bl2c = consts.tile([CPAD, 1], F32)
            nc.sync.dma_start(bl2c[:], bl2_p[:])
            iota_sb = consts.tile([P, P], BF16)
            nc.sync.dma_start(iota_sb[:], iota_p[:])

            t0loc = dram.tile([shard_pad, P], BF16)
            t0full = dram.tile([TAB, P], BF16, addr_space="Shared")
            t1loc = dram.tile([shard_pad, P], BF16)
            t1full = dram.tile([TAB, P], BF16, addr_space="Shared")
            arin = dram.tile([P, 1], F32)
            arout = dram.tile([P, 1], F32, addr_space="Shared")

            # ---- phase A: t0 shard = x_shard @ W0 (bf16) ----
            scope_in("phaseA")
            for t in range(nblk // bt):
                xsb = xin.tile([P, bt * P], BF16, tag="xsb")
                nc.sync.dma_start(
                    xsb[:], xT_p[:, t * bt * P : (t + 1) * bt * P]
                )
                for i in range(bt):
                    b = t * bt + i
                    pt0 = psA.tile([P, P], F32, space="PSUM", tag="pt0")
                    nc.tensor.matmul(
                        pt0[:], lhsT=xsb[:, i * P : (i + 1) * P], rhs=w0bf[:],
                        start=True, stop=True,
                    )
                    st = stg.tile([P, P], BF16, tag="st")
                    nc.vector.tensor_copy(st[:], pt0[:])
                    nc.sync.dma_start(t0loc[b * P : (b + 1) * P, :], st[:])

            scope_out("phaseA")
            scope_in("AG0")
            nc.gpsimd.collective_compute(
                "AllGather", mybir.AluOpType.bypass, replica_groups=rg,
                ins=[t0loc.opt()], outs=[t0full.opt()],
            )
            scope_out("AG0")

            def agg_layer(tfull, bias_col, produce, acc_tile):
                """Aggregate over this core's dst blocks, gathering from tfull.
                produce=True: write relu-layer t-transform into t1loc.
                produce=False: accumulate readout sums into acc_tile."""
                # first/last chunk position of each block (for psum start/stop)
                for gi, blocks in enumerate(groups):
                    c0, ncols = bg_coloff[gi]
                    nch = sum(int(cap[b, q]) for b in blocks
                              for q in range(PIECES)) // P
                    itile = idxp.tile([128, IDXW], I16, tag="isb")
                    nc.sync.dma_start(itile[:, :ncols], gidx_p[:, c0 : c0 + ncols])
                    swt = swp.tile([128, 2 * NCHW], F32, tag="swb")
                    o = 2 * ch_off[gi]
                    nc.sync.dma_start(swt[:, : 2 * nch], gsw_p[:, o : o + 2 * nch])
                    ssb = swt[:, :nch]
                    wsb = swt[:, nch : 2 * nch]

                    pagg = {}
                    for b in blocks:
                        pagg[b] = psB.tile([P, P], F32, space="PSUM",
                                           tag="pagg", name=f"pagg_{b}")
                    nchunks_of = {
                        b: sum(int(cap[b, q]) for q in range(PIECES)) // P
                        for b in blocks
                    }
                    seen = {b: 0 for b in blocks}

                    ci = 0  # chunk counter within bg (stream order)
                    for q in range(PIECES):
                        cq = call_cap[(gi, q)]
                        assert cq > 0, "empty gather call breaks queue-lane alignment"

                        G = gp.tile([P, GMAX], BF16, tag="G")
                        g3 = G[:, :cq].rearrange("p (c e) -> p c e", e=P)
                        qo = call_coloff[(gi, q)]
                        nc.gpsimd.dma_gather(
                            g3,
                            tfull[q * prows : (q + 1) * prows, :],
                            itile[:, qo : qo + cq // 16],
                            cq,
                            cq,
                            P,
                            queue_num=q,
                            single_packet=False,
                        )
                        off = 0
                        for b in blocks:
                            nbq = int(cap[b, q]) // P
                            for j in range(nbq):
                                S = sp.tile([P, P], BF16, tag="S")
                                nc.vector.tensor_scalar(
                                    out=S[:], in0=iota_sb[:],
                                    scalar1=ssb[:, ci : ci + 1],
                                    scalar2=wsb[:, ci : ci + 1],
                                    op0=mybir.AluOpType.is_equal,
                                    op1=mybir.AluOpType.mult,
                                )
                                nc.tensor.matmul(
                                    pagg[b][:],
                                    lhsT=G[:, (off + j) * P : (off + j + 1) * P],
                                    rhs=S[:],
                                    start=(seen[b] == 0),
                                    stop=(seen[b] == nchunks_of[b] - 1),
                                )
                                seen[b] += 1
                                ci += 1
                            off += nbq
                    assert ci == nch

                    for b in blocks:
                        assert seen[b] == nchunks_of[b] and nchunks_of[b] > 0
                        if produce:
                            h1 = hp.tile([P, P], BF16, tag="h1")
                            nc.scalar.activation(
                                out=h1[:], in_=pagg[b][:],
                                func=mybir.ActivationFunctionType.Relu,
                                bias=bias_col,
                            )
                            pt1 = psA.tile([P, P], F32, space="PSUM", tag="pt0")
                            nc.tensor.matmul(
                                pt1[:], lhsT=h1[:], rhs=w1bf[:],
                                start=True, stop=True,
                            )
                            st1 = stg.tile([P, P], BF16, tag="st")
                            nc.vector.tensor_copy(st1[:], pt1[:])
                            nc.sync.dma_start(t1loc[b * P : (b + 1) * P, :], st1[:])
                        else:
                            nv = valid_last if b == nblk - 1 else P
                            dead = hp.tile([P, P], BF16, tag="h1")
                            nc.scalar.activation(
                                out=dead[:, :nv], in_=pagg[b][:, :nv],
                                func=mybir.ActivationFunctionType.Relu,
                                bias=bias_col,
                                accum_out=acc_tile[:, b : b + 1],
                            )

            scope_in("aggL1")
            agg_layer(t0full, b0c[:, 0:1], True, None)
            scope_out("aggL1")

            scope_in("AG1")
            nc.gpsimd.collective_compute(
                "AllGather", mybir.AluOpType.bypass, replica_groups=rg,
                ins=[t1loc.opt()], outs=[t1full.opt()],
            )
            scope_out("AG1")

            acc = misc.tile([P, nblk], F32)
            scope_in("aggL2")
            agg_layer(t1full, b1c[:, 0:1], False, acc)
            scope_out("aggL2")

            # ---- readout: mean-pool + tiny MLP head (column form) ----
            partial = misc.tile([P, 1], F32)
            nc.vector.tensor_reduce(
                out=partial[:], in_=acc[:], axis=mybir.AxisListType.X,
                op=mybir.AluOpType.add,
            )
            nc.sync.dma_start(arin[:], partial[:])
            nc.gpsimd.collective_compute(
                "AllReduce", mybir.AluOpType.add, replica_groups=rg,
                ins=[arin.opt()], outs=[arout.opt()],
            )
            mr = misc.tile([P, 1], F32)
            nc.sync.dma_start(mr[:], arout[:])
            mc = misc.tile([P, 1], F32)
            nc.vector.tensor_scalar_mul(mc[:], mr[:], 1.0 / float(N))
            ph = psH.tile([P, 1], F32, space="PSUM", tag="ph")
            nc.tensor.matmul(ph[:], lhsT=wl1sb[:], rhs=mc[:], start=True, stop=True)
            z = misc.tile([P, 1], F32)
            nc.vector.tensor_scalar(
                out=z[:], in0=ph[:], scalar1=bl1c[:, 0:1], scalar2=None,
                op0=mybir.AluOpType.add,
            )
            za = misc.tile([P, 1], F32)
            nc.vector.tensor_scalar_mul(za[:], z[:], LEAKY)
            hg = misc.tile([P, 1], F32)
            nc.vector.tensor_tensor(
                out=hg[:], in0=z[:], in1=za[:], op=mybir.AluOpType.max
            )
            po = psH.tile([P, 1], F32, space="PSUM", tag="ph")
            nc.tensor.matmul(
                po[:CPAD, :], lhsT=wl2sb[:], rhs=hg[:], start=True, stop=True
            )
            yv = misc.tile([CPAD, 1], F32)
            nc.vector.tensor_scalar(
                out=yv[:], in0=po[:CPAD, :], scalar1=bl2c[:, 0:1], scalar2=None,
                op0=mybir.AluOpType.add,
            )
            nc.sync.dma_start(y_p[:], yv[:])

    nc.compile()
    return nc


def make_in_maps(hd, W0, b0, W1, b1, WL1, bL1, WL2, bL2):
    C = WL2.shape[1]
    wl2p = np.zeros((P, CPAD), np.float32)
    wl2p[:, :C] = np.asarray(WL2, np.float32)
    bl2c = np.zeros((CPAD, 1), np.float32)
    bl2c[:C, 0] = np.asarray(bL2, np.float32)
    shared = dict(
        W0=np.asarray(W0, np.float32).astype(ml_dtypes.bfloat16),
        W1=np.asarray(W1, np.float32).astype(ml_dtypes.bfloat16),
        WL1=np.asarray(WL1, np.float32), WL2=wl2p,
        b0=np.asarray(b0, np.float32).reshape(P, 1),
        b1=np.asarray(b1, np.float32).reshape(P, 1),
        bL1=np.asarray(bL1, np.float32).reshape(P, 1), bL2=bl2c,
        iota=hd["iota"],
    )
    return [
        dict(shared, xT=hd["xT"][k], gidx=hd["gidx"][k], gsw=hd["gsw"][k])
        for k in range(NCORES)
    ]


def kernel(x, src, dst, W0, b0, W1, b1, WL1, bL1, WL2, bL2):
    x = np.asarray(x)
    hd = preprocess(x, np.asarray(src), np.asarray(dst))
    nc = build_nc(hd["meta"])
    in_maps = make_in_maps(hd, W0, b0, W1, b1, WL1, bL1, WL2, bL2)
    res = run_bass_kernel_spmd(nc, in_maps, list(range(NCORES)))
    C = np.asarray(WL2).shape[1]
    return res.results[0]["y"][:C, 0].reshape(1, C).astype(np.float32)


# revision 12
# speedup vs baseline: 1.1640x; 1.1141x over previous
"""Distributed 2-layer GCN (DGL GraphConv norm='both') + mean-pool head on 8 TRN2 NeuronCores.

Strategy
--------
GraphConv is linear, so fold both degree normalizations into per-edge weights
w_e = rsqrt(deg_out[src_e]) * rsqrt(deg_in[dst_e]) and reorder each layer as
transform-then-aggregate:

    t = x @ W                       (dense matmul, node-sharded across cores)
    agg[d] = sum_e w_e * t[src_e]   (sparse aggregation, dst-sharded)
    feat = relu(agg + b)            (leaky_relu after relu is a no-op)

Sharding: nodes are split into 8 contiguous shards; core k computes t-rows and
aggregations for its shard. After each transform the t-shards are AllGathered
so every core can gather arbitrary source rows.

Aggregation on device: edges are sorted by dst and grouped into 128-dst-node
blocks (host-side, index-only preprocessing). Source rows are fetched with
BULK dma_gather (the SWDGE gather custom-DMA: thousands of int16 indices per
call, 256B rows) instead of per-chunk indirect DMAs. Because gather indices
are int16 (< 32768), the 100352-row t-table is split into 4 pieces of 25088
rows; each (dst-block, piece) edge segment is padded to a multiple of 128 and
gathers are batched over groups of 4 dst blocks -> ~100 gather calls per
layer per core instead of ~3300 indirect DMAs. Within each segment edges are
sorted by source row so the random 256B reads walk ascending addresses.

For each chunk of 128 edges, the scatter-add becomes a dense matmul: build
S[e, slot] = (iota[slot] == dst_slot_e) * w_e with one fused tensor_scalar op,
then PSUM-accumulate G_chunk.T @ S -> agg^T [feat, dst_slot]. The transposed
layout makes the bias a per-partition ACT bias and feeds the next layer's
matmul (lhsT = relu(agg^T + b)) with no transposes anywhere.

Readout: ACT accum_out gives the free-axis (dst) sum of relu() for free;
per-core partials are AllReduced, and the tiny MLP head runs in column form
(out = W.T @ col) on every core redundantly.
"""

import sys

sys.path.insert(0, "/opt/trn_rl_repo")

import numpy as np
import ml_dtypes

import concourse.bacc as bacc
import concourse.bass as bass
import concourse.mybir as mybir
import concourse.tile as tile
from concourse.bass_utils import run_bass_kernel_spmd

NCORES = 8
NO_COLL = False  # timing-bisect knob: replace collectives with local copies
P = 128
CPAD = 16
PIECES = 4
BGS = 2  # dst blocks per gather batch
LEAKY = 0.01
F32 = mybir.dt.float32
BF16 = mybir.dt.bfloat16
I16 = mybir.dt.int16


def preprocess(x, src, dst):
    """Index-only host preprocessing: degree rsqrt folding, edge partitioning
    by (dst-core, dst-block, src-piece), per-(block,piece) padding to 128
    multiples (shared caps across cores for SPMD), per-core input streams."""
    N, D = x.shape
    E = src.shape[0]
    assert D == P and N % NCORES == 0
    shard = N // NCORES
    nblk = -(-shard // P)
    shard_pad = nblk * P
    valid_last = shard - (nblk - 1) * P
    TAB = NCORES * shard_pad
    assert TAB % PIECES == 0
    prows = TAB // PIECES
    assert prows <= 32768

    src = np.asarray(src).astype(np.int64)
    dst = np.asarray(dst).astype(np.int64)

    deg_out = np.bincount(src, minlength=N).astype(np.float32)
    deg_in = np.bincount(dst, minlength=N).astype(np.float32)
    r_out = (1.0 / np.sqrt(np.maximum(deg_out, 1.0))).astype(np.float32)
    r_in = (1.0 / np.sqrt(np.maximum(deg_in, 1.0))).astype(np.float32)

    pid = ((src // shard) * shard_pad + (src % shard)).astype(np.int64)
    core = dst // shard
    local = dst % shard
    block = local // P
    slot = (local % P).astype(np.float32)
    piece = pid // prows
    idxrel = (pid % prows).astype(np.int64)
    w = (r_out[src] * r_in[dst]).astype(np.float32)

    # sort edges by (core, block, piece, idxrel)
    key = ((core * nblk + block) * PIECES + piece) * np.int64(prows) + idxrel
    order = np.argsort(key, kind="stable")
    core_s = core[order]
    gkey = (core * nblk + block) * PIECES + piece
    counts = np.bincount(gkey, minlength=NCORES * nblk * PIECES).reshape(
        NCORES, nblk, PIECES
    )
    cap = (-(-counts.max(axis=0) // P) * P).astype(np.int64)  # [nblk, PIECES]
    assert (counts.max(axis=0) <= cap).all()

    # block groups
    groups = [list(range(i, min(i + BGS, nblk))) for i in range(0, nblk, BGS)]

    # stream layout per core: for bg: for q: for b in bg: segment cap[b][q]
    seg_off = np.zeros((nblk, PIECES), np.int64)
    pos = 0
    for blocks in groups:
        for q in range(PIECES):
            for b in blocks:
                seg_off[b, q] = pos
                pos += cap[b, q]
    TOT = pos
    assert TOT % P == 0
    TOTCH = TOT // P

    # rank of each edge within its (core, block, piece) segment
    gkey_s = gkey[order]
    # edges sorted by key -> within-segment rank = arange - segment start
    seg_starts_sorted = np.zeros(NCORES * nblk * PIECES, np.int64)
    cnt_flat = np.bincount(gkey_s, minlength=NCORES * nblk * PIECES)
    seg_starts_sorted[1:] = np.cumsum(cnt_flat)[:-1]
    rank = np.arange(E, dtype=np.int64) - seg_starts_sorted[gkey_s]
    b_s = block[order]
    q_s = piece[order]
    stream_pos = seg_off[b_s, q_s] + rank  # position within the core's stream

    idx_stream = np.zeros((NCORES, TOT), np.int16)
    slot_stream = np.zeros((NCORES, TOT), np.float32)
    w_stream = np.zeros((NCORES, TOT), np.float32)
    idx_stream[core_s, stream_pos] = idxrel[order].astype(np.int16)
    slot_stream[core_s, stream_pos] = slot[order]
    w_stream[core_s, stream_pos] = w[order]

    # gidx: wrap each call segment [16, len/16] -> replicate x8 -> [128, ...]
    # wrapped element g -> [g%16, g//16]; since every call segment length is a
    # multiple of 128 (hence 16), wrapping the whole stream at once is
    # identical to wrapping per call.
    gidx = np.zeros((NCORES, 128, TOT // 16), np.int16)
    w16 = idx_stream.reshape(NCORES, TOT // 16, 16).transpose(0, 2, 1)
    gidx[:] = np.tile(w16, (1, 8, 1))

    # per-chunk slot/w arrays [128, TOTCH]
    sarr = slot_stream.reshape(NCORES, TOTCH, P).transpose(0, 2, 1)
    warr = w_stream.reshape(NCORES, TOTCH, P).transpose(0, 2, 1)
    # gsw layout: per bg: [slot chunks | w chunks]
    sw_parts = []
    ch_off = {}
    c = 0
    for gi, blocks in enumerate(groups):
        nch = sum(int(cap[b, q]) for b in blocks for q in range(PIECES)) // P
        ch_off[gi] = c
        sw_parts.append(sarr[:, :, c : c + nch])
        sw_parts.append(warr[:, :, c : c + nch])
        c += nch
    assert c == TOTCH
    gsw = np.concatenate(sw_parts, axis=2)

    xT = np.zeros((NCORES, P, shard_pad), ml_dtypes.bfloat16)
    xv = np.ascontiguousarray(np.asarray(x, np.float32))
    for k in range(NCORES):
        xT[k, :, :shard] = xv[k * shard : (k + 1) * shard].T.astype(
            ml_dtypes.bfloat16)

    iota = np.tile(np.arange(P, dtype=np.float32), (P, 1)).astype(
        ml_dtypes.bfloat16)

    bt = max(d for d in range(1, 33) if nblk % d == 0)

    meta = dict(
        N=N, E=E, shard=shard, nblk=nblk, shard_pad=shard_pad, bt=bt,
        valid_last=valid_last, TAB=TAB, prows=prows, TOT=TOT, TOTCH=TOTCH,
        cap=cap, groups=groups, ch_off=ch_off,
    )
    return dict(meta=meta, xT=xT, gidx=gidx, gsw=gsw, iota=iota)


def build_nc(meta):
    """Build the SPMD Bass program (same program for all 8 cores)."""
    nblk = meta["nblk"]
    shard_pad = meta["shard_pad"]
    bt = meta["bt"]
    valid_last = meta["valid_last"]
    TAB = meta["TAB"]
    prows = meta["prows"]
    TOT = meta["TOT"]
    TOTCH = meta["TOTCH"]
    cap = meta["cap"]
    groups = meta["groups"]
    ch_off = meta["ch_off"]
    N = meta["N"]

    # per-call capacities and offsets
    call_cap = {}  # (gi, q) -> num idxs
    call_coloff = {}  # (gi, q) -> col offset into gidx
    bg_coloff = {}  # gi -> (col0, ncols)
    pos = 0
    for gi, blocks in enumerate(groups):
        c0 = pos // 16
        for q in range(PIECES):
            call_cap[(gi, q)] = sum(int(cap[b, q]) for b in blocks)
            call_coloff[(gi, q)] = (pos // 16) - c0
            pos += call_cap[(gi, q)]
        bg_coloff[gi] = (c0, pos // 16 - c0)
    GMAX = max(call_cap.values())
    IDXW = max(n for _, n in bg_coloff.values())
    NCHW = max(
        sum(int(cap[b, q]) for b in blocks for q in range(PIECES)) // P
        for blocks in groups
    )

    rg = [list(range(NCORES))]
    nc = bacc.Bacc("TRN2", target_bir_lowering=False, debug=False,
                   num_devices=NCORES, dynamic_dma_scratch_size=32768,
                   num_swdge_queues=4)

    xT_p = nc.declare_dram_parameter("xT", [P, shard_pad], BF16, False)
    w0_p = nc.declare_dram_parameter("W0", [P, P], BF16, False)
    w1_p = nc.declare_dram_parameter("W1", [P, P], BF16, False)
    wl1_p = nc.declare_dram_parameter("WL1", [P, P], F32, False)
    wl2_p = nc.declare_dram_parameter("WL2", [P, CPAD], F32, False)
    b0_p = nc.declare_dram_parameter("b0", [P, 1], F32, False)
    b1_p = nc.declare_dram_parameter("b1", [P, 1], F32, False)
    bl1_p = nc.declare_dram_parameter("bL1", [P, 1], F32, False)
    bl2_p = nc.declare_dram_parameter("bL2", [CPAD, 1], F32, False)
    iota_p = nc.declare_dram_parameter("iota", [P, P], BF16, False)
    gidx_p = nc.declare_dram_parameter("gidx", [128, TOT // 16], I16, False)
    gsw_p = nc.declare_dram_parameter("gsw", [128, 2 * TOTCH], F32, False)
    y_p = nc.declare_dram_parameter("y", [CPAD, 1], F32, True)

    _sids = {}

    def scope_in(name):
        _sids[name] = nc.enter_named_scope(name, False)[0]

    def scope_out(name):
        nc.leave_named_scope(name, _sids[name], False)

    with tile.TileContext(nc) as tc:
        with (
            tc.tile_pool(name="consts", bufs=1) as consts,
            tc.tile_pool(name="xin", bufs=2) as xin,
            tc.tile_pool(name="stg", bufs=3) as stg,
            tc.tile_pool(name="idxp", bufs=2) as idxp,
            tc.tile_pool(name="swp", bufs=2) as swp,
            tc.tile_pool(name="gp", bufs=3) as gp,
            tc.tile_pool(name="sp", bufs=4) as sp,
            tc.tile_pool(name="hp", bufs=2) as hp,
            tc.tile_pool(name="misc", bufs=1) as misc,
            tc.tile_pool(name="psA", bufs=2, space="PSUM") as psA,
            tc.tile_pool(name="psB", bufs=4, space="PSUM") as psB,
            tc.tile_pool(name="psH", bufs=2, space="PSUM") as psH,
            tc.tile_pool(name="dram", bufs=1, space="DRAM") as dram,
        ):
            # ---- constants ----
            w0bf = consts.tile([P, P], BF16)
            nc.sync.dma_start(w0bf[:], w0_p[:])
            w1bf = consts.tile([P, P], BF16)
            nc.sync.dma_start(w1bf[:], w1_p[:])
            wl1sb = consts.tile([P, P], F32)
            nc.sync.dma_start(wl1sb[:], wl1_p[:])
            wl2sb = consts.tile([P, CPAD], F32)
            nc.sync.dma_start(wl2sb[:], wl2_p[:])
            b0c = consts.tile([P, 1], F32)
            nc.sync.dma_start(b0c[:], b0_p[:])
            b1c = consts.tile([P, 1], F32)
            nc.sync.dma_start(b1c[:], b1_p[:])
            bl1c = consts.tile([P, 1], F32)
            nc.sync.dma_start(bl1c[:], bl1_p[:])
            bl2c = consts.tile([CPAD, 1], F32)
            nc.sync.dma_start(bl2c[:], bl2_p[:])
            iota_sb = consts.tile([P, P], BF16)
            nc.sync.dma_start(iota_sb[:], iota_p[:])

            shr = {} if NO_COLL else {"addr_space": "Shared"}
            t0loc = dram.tile([shard_pad, P], BF16)
            t0full = dram.tile([TAB, P], BF16, **shr)
            t1loc = dram.tile([shard_pad, P], BF16)
            t1full = dram.tile([TAB, P], BF16, **shr)
            arin = dram.tile([P, 1], F32)
            arout = dram.tile([P, 1], F32, **shr)

            # ---- phase A: t0 shard = x_shard @ W0 (bf16) ----
            scope_in("phaseA")
            for t in range(nblk // bt):
                xsb = xin.tile([P, bt * P], BF16, tag="xsb")
                nc.sync.dma_start(
                    xsb[:], xT_p[:, t * bt * P : (t + 1) * bt * P]
                )
                for i in range(bt):
                    b = t * bt + i
                    pt0 = psA.tile([P, P], F32, space="PSUM", tag="pt0")
                    nc.tensor.matmul(
                        pt0[:], lhsT=xsb[:, i * P : (i + 1) * P], rhs=w0bf[:],
                        start=True, stop=True,
                    )
                    st = stg.tile([P, P], BF16, tag="st")
                    nc.vector.tensor_copy(st[:], pt0[:])
                    nc.sync.dma_start(t0loc[b * P : (b + 1) * P, :], st[:])

            scope_out("phaseA")
            scope_in("AG0")
            if NO_COLL:
                for k in range(NCORES):
                    nc.sync.dma_start(
                        t0full[k * shard_pad : (k + 1) * shard_pad, :], t0loc[:])
            else:
                nc.gpsimd.collective_compute(
                    "AllGather", mybir.AluOpType.bypass, replica_groups=rg,
                    ins=[t0loc.opt()], outs=[t0full.opt()],
                )
            scope_out("AG0")

            def agg_layer(tfull, bias_col, produce, acc_tile):
                """Aggregate over this core's dst blocks, gathering from tfull.
                produce=True: write relu-layer t-transform into t1loc.
                produce=False: accumulate readout sums into acc_tile."""
                # first/last chunk position of each block (for psum start/stop)
                for gi, blocks in enumerate(groups):
                    c0, ncols = bg_coloff[gi]
                    nch = sum(int(cap[b, q]) for b in blocks
                              for q in range(PIECES)) // P
                    itile = idxp.tile([128, IDXW], I16, tag="isb")
                    nc.sync.dma_start(itile[:, :ncols], gidx_p[:, c0 : c0 + ncols])
                    swt = swp.tile([128, 2 * NCHW], F32, tag="swb")
                    o = 2 * ch_off[gi]
                    nc.sync.dma_start(swt[:, : 2 * nch], gsw_p[:, o : o + 2 * nch])
                    ssb = swt[:, :nch]
                    wsb = swt[:, nch : 2 * nch]

                    pagg = {}
                    for b in blocks:
                        pagg[b] = psB.tile([P, P], F32, space="PSUM",
                                           tag="pagg", name=f"pagg_{b}")
                    nchunks_of = {
                        b: sum(int(cap[b, q]) for q in range(PIECES)) // P
                        for b in blocks
                    }
                    seen = {b: 0 for b in blocks}

                    ci = 0  # chunk counter within bg (stream order)
                    for q in range(PIECES):
                        cq = call_cap[(gi, q)]
                        assert cq > 0, "empty gather call breaks queue-lane alignment"

                        G = gp.tile([P, GMAX], BF16, tag="G")
                        g3 = G[:, :cq].rearrange("p (c e) -> p c e", e=P)
                        qo = call_coloff[(gi, q)]
                        nc.gpsimd.dma_gather(
                            g3,
                            tfull[q * prows : (q + 1) * prows, :],
                            itile[:, qo : qo + cq // 16],
                            cq,
                            cq,
                            P,
                            queue_num=q,
                            single_packet=False,
                        )
                        off = 0
                        for b in blocks:
                            nbq = int(cap[b, q]) // P
                            for j in range(nbq):
                                S = sp.tile([P, P], BF16, tag="S")
                                nc.vector.tensor_scalar(
                                    out=S[:], in0=iota_sb[:],
                                    scalar1=ssb[:, ci : ci + 1],
                                    scalar2=wsb[:, ci : ci + 1],
                                    op0=mybir.AluOpType.is_equal,
                                    op1=mybir.AluOpType.mult,
                                )
                                nc.tensor.matmul(
                                    pagg[b][:],
                                    lhsT=G[:, (off + j) * P : (off + j + 1) * P],
                                    rhs=S[:],
                                    start=(seen[b] == 0),
                                    stop=(seen[b] == nchunks_of[b] - 1),
                                )
                                seen[b] += 1
                                ci += 1
                            off += nbq
                    assert ci == nch

                    for b in blocks:
                        assert seen[b] == nchunks_of[b] and nchunks_of[b] > 0
                        if produce:
                            h1 = hp.tile([P, P], BF16, tag="h1")
                            nc.scalar.activation(
                                out=h1[:], in_=pagg[b][:],
                                func=mybir.ActivationFunctionType.Relu,
                                bias=bias_col,
                            )
                            pt1 = psA.tile([P, P], F32, space="PSUM", tag="pt0")
                            nc.tensor.matmul(
                                pt1[:], lhsT=h1[:], rhs=w1bf[:],
                                start=True, stop=True,
                            )
                            st1 = stg.tile([P, P], BF16, tag="st")
                            nc.vector.tensor_copy(st1[:], pt1[:])
                            nc.sync.dma_start(t1loc[b * P : (b + 1) * P, :], st1[:])
                        else:
                            nv = valid_last if b == nblk - 1 else P
                            dead = hp.tile([P, P], BF16, tag="h1")
                            nc.scalar.activation(
                                out=dead[:, :nv], in_=pagg[b][:, :nv],
                                func=mybir.ActivationFunctionType.Relu,
                                bias=bias_col,
                                accum_out=acc_tile[:, b : b + 1],
                            )

            scope_in("aggL1")
            agg_layer(t0full, b0c[:, 0:1], True, None)
            scope_out("aggL1")

            scope_in("AG1")
            if NO_COLL:
                for k in range(NCORES):
                    nc.sync.dma_start(
                        t1full[k * shard_pad : (k + 1) * shard_pad, :], t1loc[:])
            else:
                nc.gpsimd.collective_compute(
                    "AllGather", mybir.AluOpType.bypass, replica_groups=rg,
                    ins=[t1loc.opt()], outs=[t1full.opt()],
                )
            scope_out("AG1")

            acc = misc.tile([P, nblk], F32)
            scope_in("aggL2")
            agg_layer(t1full, b1c[:, 0:1], False, acc)
            scope_out("aggL2")

            # ---- readout: mean-pool + tiny MLP head (column form) ----
            partial = misc.tile([P, 1], F32)
            nc.vector.tensor_reduce(
                out=partial[:], in_=acc[:], axis=mybir.AxisListType.X,
                op=mybir.AluOpType.add,
            )
            nc.sync.dma_start(arin[:], partial[:])
            if NO_COLL:
                nc.sync.dma_start(arout[:], arin[:])
            else:
                nc.gpsimd.collective_compute(
                    "AllReduce", mybir.AluOpType.add, replica_groups=rg,
                    ins=[arin.opt()], outs=[arout.opt()],
                )
            mr = misc.tile([P, 1], F32)
            nc.sync.dma_start(mr[:], arout[:])
            mc = misc.tile([P, 1], F32)
            nc.vector.tensor_scalar_mul(mc[:], mr[:], 1.0 / float(N))
            ph = psH.tile([P, 1], F32, space="PSUM", tag="ph")
            nc.tensor.matmul(ph[:], lhsT=wl1sb[:], rhs=mc[:], start=True, stop=True)
            z = misc.tile([P, 1], F32)
            nc.vector.tensor_scalar(
                out=z[:], in0=ph[:], scalar1=bl1c[:, 0:1], scalar2=None,
                op0=mybir.AluOpType.add,
            )
            za = misc.tile([P, 1], F32)
            nc.vector.tensor_scalar_mul(za[:], z[:], LEAKY)
            hg = misc.tile([P, 1], F32)
            nc.vector.tensor_tensor(
                out=hg[:], in0=z[:], in1=za[:], op=mybir.AluOpType.max
            )
            po = psH.tile([P, 1], F32, space="PSUM", tag="ph")
            nc.tensor.matmul(
                po[:CPAD, :], lhsT=wl2sb[:], rhs=hg[:], start=True, stop=True
            )
            yv = misc.tile([CPAD, 1], F32)
            nc.vector.tensor_scalar(
                out=yv[:], in0=po[:CPAD, :], scalar1=bl2c[:, 0:1], scalar2=None,
                op0=mybir.AluOpType.add,
            )
            nc.sync.dma_start(y_p[:], yv[:])

    nc.compile()
    return nc


def make_in_maps(hd, W0, b0, W1, b1, WL1, bL1, WL2, bL2):
    C = WL2.shape[1]
    wl2p = np.zeros((P, CPAD), np.float32)
    wl2p[:, :C] = np.asarray(WL2, np.float32)
    bl2c = np.zeros((CPAD, 1), np.float32)
    bl2c[:C, 0] = np.asarray(bL2, np.float32)
    shared = dict(
        W0=np.asarray(W0, np.float32).astype(ml_dtypes.bfloat16),
        W1=np.asarray(W1, np.float32).astype(ml_dtypes.bfloat16),
        WL1=np.asarray(WL1, np.float32), WL2=wl2p,
        b0=np.asarray(b0, np.float32).reshape(P, 1),
        b1=np.asarray(b1, np.float32).reshape(P, 1),
        bL1=np.asarray(bL1, np.float32).reshape(P, 1), bL2=bl2c,
        iota=hd["iota"],
    )
    return [
        dict(shared, xT=hd["xT"][k], gidx=hd["gidx"][k], gsw=hd["gsw"][k])
        for k in range(NCORES)
    ]


def kernel(x, src, dst, W0, b0, W1, b1, WL1, bL1, WL2, bL2):
    x = np.asarray(x)
    hd = preprocess(x, np.asarray(src), np.asarray(dst))
    nc = build_nc(hd["meta"])
    in_maps = make_in_maps(hd, W0, b0, W1, b1, WL1, bL1, WL2, bL2)
    res = run_bass_kernel_spmd(nc, in_maps, list(range(NCORES)))
    C = np.asarray(WL2).shape[1]
    return res.results[0]["y"][:C, 0].reshape(1, C).astype(np.float32)
